# revision 19
# baseline (speedup 1.0000x reference)
"""Trainium2 Bass kernel for nn_CriticNetwork (gnn_message_passing).

Math: the reference GNN does mean-aggregation over a complete graph with
self-loops, so every node of an env sees the identical per-env mean.  The
whole network collapses to per-env scalars:

  m_b  = mean over the 16 nodes of obs[b]                      [128]
  p_b  = relu(m_b @ W1 + b1) @ W2 + b2                         [64]
  a_b  = p_b . (Wfc @ (Wattn[:64] + Wattn[64:]))               scalar
  w_b  = sigmoid(leaky_relu(a_b, 0.01))                        scalar
  c_b  = p_b . Wv[:64] + bv                                    scalar
  Q_bj = (act[b,j]-pi[b,j]) . Wvy ;  (Wvy = Wv[64:72])
  PS_b = sum_j pi[b,j].Wvy ;  QS_b = sum_j Q_bj
  xv[b,j] = c_b + (PS_b + w_b*QS_b)/16 - (w_b/16)*Q_bj
  out x[b*16+d, j] = xv[b,j]   (independent of d)
  out w[b*16+d, j] = w_b

Sharding: data-parallel over envs, 512 envs per core x 8 cores.

Per-core layout: local env e = 4*p + g (p = partition, g = group).
Groups are processed in PAIRS (pair 0 = g0,g1; pair 1 = g2,g3): a pair's
obs rows 64p+32*pair .. +31 are one contiguous 16KB HBM run per
partition, and all compute runs pair-wide, halving instruction count
(fixed ~13.7us preamble/teardown dominates; sem hops cost ~150ns each).

Engine plan (v5):
  - per pair, obs streams as two half-node DMAs (8 nodes each), one per
    HWDGE ring; only 3-4 entries per ring -> no queue-depth stalls.
  - DVE: pairwise-tree partial sums per half (overlaps the stream), +
    tiny X-reduces/scales, + pair-1 combine.
  - GpSimd: pol/act dot block via slice trees + pair-0 combine.
  - leaky-relu via DVE scalar_tensor_tensor (AFT.Lrelu would thrash the
    ACT table against Sigmoid).
  - separate output tiles per pair/engine; 4 output DMAs (2 per ring),
    2KB contiguous per partition.
"""

import numpy as np
from contextlib import ExitStack

import concourse.bass as bass
import concourse.bacc as bacc
import concourse.tile as tile
from concourse import mybir
from concourse.bass_utils import run_bass_kernel_spmd

B, N, A = 4096, 16, 8
D_IN, H1, DP, DZ = 128, 64, 64, 64
NCORES = 8
BC = B // NCORES          # 512 envs per core
RC = BC * N               # 8192 obs rows per core
G = 4                     # env groups per core
GE = BC // G              # 128 envs per group
CW = 272                  # const tile width

F32 = mybir.dt.float32
BF16 = mybir.dt.bfloat16
ALU = mybir.AluOpType
AFT = mybir.ActivationFunctionType


def _build():
    nc = bacc.Bacc("TRN2", target_bir_lowering=False, debug=False)

    obs = nc.dram_tensor("obs", [RC, D_IN], F32, kind="ExternalInput")
    pol = nc.dram_tensor("pol", [RC, A], F32, kind="ExternalInput")
    act = nc.dram_tensor("act", [RC, A], F32, kind="ExternalInput")
    cst = nc.dram_tensor("cst", [128, CW], F32, kind="ExternalInput")
    xo = nc.dram_tensor("xo", [RC, N], F32, kind="ExternalOutput")
    wo = nc.dram_tensor("wo", [RC, N], F32, kind="ExternalOutput")

    with ExitStack() as ctx:
        tc = ctx.enter_context(tile.TileContext(nc))
        consts = ctx.enter_context(tc.tile_pool(name="consts", bufs=1))
        obsp = ctx.enter_context(tc.tile_pool(name="obsp", bufs=2))
        pap = ctx.enter_context(tc.tile_pool(name="pap", bufs=1))
        gsb = ctx.enter_context(tc.tile_pool(name="gsb", bufs=1))
        sb = ctx.enter_context(tc.tile_pool(name="sb", bufs=2))
        smal = ctx.enter_context(tc.tile_pool(name="smal", bufs=2))
        outp = ctx.enter_context(tc.tile_pool(name="outp", bufs=1))
        pmtp = ctx.enter_context(tc.tile_pool(name="pmtp", bufs=2, space="PSUM"))
        php = ctx.enter_context(tc.tile_pool(name="php", bufs=2, space="PSUM"))
        pacp = ctx.enter_context(tc.tile_pool(name="pacp", bufs=2, space="PSUM"))
        pwtp = ctx.enter_context(tc.tile_pool(name="pwtp", bufs=2, space="PSUM"))

        # ring A (sync): pol, pair0-h0, act, pair1-h0;
        # ring B (scalar): cst, pair0-h1, pair1-h1
        pa_view = lambda t: t.ap().rearrange("(p g n) a -> p (g n a)",
                                             p=128, g=G, n=16)
        act_sb = pap.tile([128, G * N * A], F32)
        nc.sync.dma_start(out=act_sb, in_=pa_view(act))
        cst_sb = consts.tile([128, CW], F32)
        nc.scalar.dma_start(out=cst_sb, in_=cst.ap())
        pol_sb = pap.tile([128, G * N * A], F32)
        nc.scalar.dma_start(out=pol_sb, in_=pa_view(pol))

        wvy8_sb = cst_sb[:, 0:8]            # Wvy on all partitions
        w1q_sb = cst_sb[:, 8:72]            # W1 / 16
        wq_sb = cst_sb[0:64, 72:74]         # W2 @ [wa | Wv[:64]]
        b1_sb = cst_sb[0:64, 138:139]
        biasq_sb = cst_sb[0:2, 140:141]     # [b2.wa, b2.Wv64 + bv]
        id2_sb = cst_sb[0:2, 142:144]       # eye(2)
        id128_sb = cst_sb[:, 144:272]       # eye(128)

        # obs row = 64p + 32pr + 16g2 + 8h + nf; pair tile free layout
        # (h, g2, nf, f): each half h is (g2 nf f) = 2048 contiguous
        obs_v = obs.ap().rearrange("(p pr g2 h nf) f -> pr h p g2 (nf f)",
                                   p=128, pr=2, g2=2, h=2, nf=8)
        pair_tiles = []
        for pr in range(2):
            t = obsp.tile([128, 4096], F32, name="pair_t")
            nc.sync.dma_start(
                out=t[:, 0:2048].rearrange("p (g2 x) -> p g2 x", g2=2),
                in_=obs_v[pr][0])
            nc.scalar.dma_start(
                out=t[:, 2048:4096].rearrange("p (g2 x) -> p g2 x", g2=2),
                in_=obs_v[pr][1])
            pair_tiles.append(t)

        # preload the sigmoid ACT table while DMAs stream
        warm = consts.tile([1, 1], F32)
        nc.scalar.activation(out=warm, in_=cst_sb[0:1, 0:1], func=AFT.Sigmoid)
        # bf16 copies of the chain constants (trees/chain run in bf16)
        w1q_bf = consts.tile([128, 64], BF16)
        nc.scalar.activation(out=w1q_bf, in_=w1q_sb, func=AFT.Copy)
        wq_bf = consts.tile([64, 2], BF16)
        nc.scalar.activation(out=wq_bf, in_=wq_sb, func=AFT.Copy)
        id128_bf = consts.tile([128, 128], BF16)
        nc.scalar.activation(out=id128_bf, in_=id128_sb, func=AFT.Copy)

        # ---- pol/act dot block on DVE: it is idle until the first obs
        # half lands, and pol/act stream first on the rings ----
        d8 = gsb.tile([128, G * N * A], F32)
        nc.vector.tensor_sub(d8, act_sb, pol_sb)
        dw = gsb.tile([128, G * N, A], F32)
        nc.vector.tensor_mul(dw, d8.rearrange("p (gr a) -> p gr a", a=A),
                             wvy8_sb.unsqueeze(1).broadcast_to([128, G * N, A]))
        Q64 = gsb.tile([128, G * N], F32)
        nc.vector.reduce_sum(out=Q64, in_=dw, axis=mybir.AxisListType.X)
        PS8 = gsb.tile([128, G, A], F32)
        nc.vector.reduce_sum(out=PS8, in_=pol_sb.rearrange(
            "p (g r a) -> p g a r", g=G, a=A), axis=mybir.AxisListType.X)
        PSw = gsb.tile([128, G, A], F32)
        nc.vector.tensor_mul(PSw, PS8,
                             wvy8_sb.unsqueeze(1).broadcast_to([128, G, A]))

        # tiny final reduces + pre-scales on DVE
        PS4 = gsb.tile([128, G], F32)
        nc.vector.reduce_sum(out=PS4, in_=PSw, axis=mybir.AxisListType.X)
        QS4 = gsb.tile([128, G], F32)
        nc.vector.reduce_sum(out=QS4,
                             in_=Q64.rearrange("p (g r) -> p g r", g=G),
                             axis=mybir.AxisListType.X)
        PS4s = gsb.tile([128, G], F32)
        nc.vector.tensor_scalar_mul(PS4s, PS4, 1.0 / N)
        QS4s = gsb.tile([128, G], F32)
        nc.vector.tensor_scalar_mul(QS4s, QS4, 1.0 / N)
        Q64n = gsb.tile([128, G * N], F32)
        nc.vector.tensor_scalar_mul(Q64n, Q64, -1.0 / N)

        # output payload tiles, one pair each (independent writers)
        wbigs = [outp.tile([128, 2 * N * N], F32, name=f"wbig{i}")
                 for i in range(2)]
        xbigs = [outp.tile([128, 2 * N * N], F32, name=f"xbig{i}")
                 for i in range(2)]

        def head(pr):
            """pair-wide: per-half trees + transpose + MLP chain.
            Returns [128,4] (w0,c0,w1,c1) per-env scalars."""
            t = pair_tiles[pr]
            va = t[:, 0:2048].rearrange("p (g x) -> p g x", g=2)
            vb = t[:, 2048:4096].rearrange("p (g x) -> p g x", g=2)
            sb2 = sb.tile([128, 2, 512], BF16, name="sb2")
            nc.vector.tensor_add(sb2, vb[:, :, 0:512], vb[:, :, 512:1024])
            sb3 = sb.tile([128, 2, 256], BF16, name="sb3")
            nc.vector.tensor_add(sb3, sb2[:, :, 0:256], sb2[:, :, 256:512])
            sb4 = sb.tile([128, 2, 128], BF16, name="sb4")
            nc.vector.tensor_add(sb4, sb3[:, :, 0:128], sb3[:, :, 128:256])
            sa2 = sb.tile([128, 2, 512], BF16, name="sa2")
            nc.vector.tensor_add(sa2, va[:, :, 0:512], va[:, :, 512:1024])
            sa3 = sb.tile([128, 2, 256], BF16, name="sa3")
            nc.vector.tensor_add(sa3, sa2[:, :, 0:256], sa2[:, :, 256:512])
            sa4 = sb.tile([128, 2, 128], BF16, name="sa4")
            nc.vector.tensor_add(sa4, sa3[:, :, 0:128], sa3[:, :, 128:256])
            meanS = sb.tile([128, 256], BF16, name="meanS")
            nc.vector.tensor_add(meanS.rearrange("p (g f) -> p g f", g=2),
                                 sa4, sb4)

            pmt = pmtp.tile([128, 256], BF16, name="pmt")
            nc.tensor.transpose(pmt[:, 0:128], meanS[:, 0:128], id128_bf)
            nc.tensor.transpose(pmt[:, 128:256], meanS[:, 128:256], id128_bf)
            meanT = sb.tile([128, 2 * GE], BF16, name="meanT")
            nc.scalar.activation(out=meanT, in_=pmt, func=AFT.Copy)
            ph = php.tile([64, 2 * GE], F32, name="ph")
            nc.tensor.matmul(ph, lhsT=w1q_bf[:], rhs=meanT[:], start=True,
                             stop=True)
            h_sb = sb.tile([64, 2 * GE], BF16, name="h_sb")
            nc.scalar.activation(out=h_sb, in_=ph, func=AFT.Relu, bias=b1_sb)
            pac = pacp.tile([2, 2 * GE], F32, name="pac")
            nc.tensor.matmul(pac, lhsT=wq_bf[:], rhs=h_sb, start=True, stop=True)
            wc = sb.tile([2, 2 * GE], F32, name="wc")
            nc.scalar.activation(out=wc, in_=pac, func=AFT.Identity,
                                 bias=biasq_sb)
            pwt = pwtp.tile([128, 4], F32, name="pwt")
            nc.tensor.transpose(pwt[:, 0:2], wc[:, 0:128], id2_sb)
            nc.tensor.transpose(pwt[:, 2:4], wc[:, 128:256], id2_sb)
            # leaky-relu + sigmoid AFTER the transpose: 128-lane-wide ops
            # on [128,2] strided column views instead of single-lane rows
            wlr = sb.tile([128, 4], F32, name="wlr")
            nc.scalar.activation(out=wlr, in_=pwt, func=AFT.Copy)
            wl = sb.tile([128, 4], F32, name="wl")
            wr4 = wlr.rearrange("p (g two) -> p g two", two=2)
            wl4 = wl.rearrange("p (g two) -> p g two", two=2)
            nc.vector.scalar_tensor_tensor(out=wl4[:, :, 0:1],
                                           in0=wr4[:, :, 0:1], scalar=0.01,
                                           in1=wr4[:, :, 0:1], op0=ALU.mult,
                                           op1=ALU.max)
            nc.scalar.activation(out=wl4[:, :, 0:1], in_=wl4[:, :, 0:1],
                                 func=AFT.Sigmoid)
            # c columns stay in wlr; w columns live in wl
            return wl, wlr

        def combine(eng, cpeng, pr, wc4, wbig, xbig):
            """pair-wide combine on `eng` + output broadcast on `cpeng`;
            tensor_tensor/copy only, so both run on GpSimd too."""
            wl, wlr = wc4
            w2 = wl.rearrange("p (g two) -> p g two", two=2)[:, :, 0:1]
            c2 = wlr.rearrange("p (g two) -> p g two", two=2)[:, :, 1:2]
            t2 = smal.tile([128, 2, 1], F32, name="t2")
            eng.tensor_mul(t2, w2,
                           QS4s[:, 2 * pr:2 * pr + 2].unsqueeze(2))
            t3 = smal.tile([128, 2, 1], F32, name="t3")
            eng.tensor_add(t3, t2,
                           PS4s[:, 2 * pr:2 * pr + 2].unsqueeze(2))
            base = smal.tile([128, 2, 1], F32, name="base")
            eng.tensor_add(base, t3, c2)
            nwq = smal.tile([128, 2, N], F32, name="nwq")
            eng.tensor_mul(nwq,
                           Q64n[:, 32 * pr:32 * (pr + 1)].rearrange(
                               "p (g r) -> p g r", g=2),
                           w2.broadcast_to([128, 2, N]))
            xv = smal.tile([128, 2, N], F32, name="xv")
            eng.tensor_add(xv, nwq, base.broadcast_to([128, 2, N]))
            cpeng.tensor_copy(wbig.rearrange("p (g dj) -> p g dj", g=2),
                              w2.broadcast_to([128, 2, 256]))
            cpeng.tensor_copy(
                xbig.rearrange("p (g d j) -> p g d j", g=2, d=16),
                xv.unsqueeze(2).broadcast_to([128, 2, 16, 16]))

        wc01 = head(0)
        wc23 = head(1)
        combine(nc.gpsimd, nc.gpsimd, 0, wc01, wbigs[0], xbigs[0])
        combine(nc.vector, nc.vector, 1, wc23, wbigs[1], xbigs[1])

        # outputs: rows (p, pr, g2, d); 2KB contiguous per partition
        wo_v = wo.ap().rearrange("(p h g2 d) j -> h p (g2 d j)",
                                 p=128, h=2, g2=2, d=16)
        xo_v = xo.ap().rearrange("(p h g2 d) j -> h p (g2 d j)",
                                 p=128, h=2, g2=2, d=16)
        nc.sync.dma_start(out=wo_v[0], in_=wbigs[0])
        nc.scalar.dma_start(out=xo_v[0], in_=xbigs[0])
        nc.scalar.dma_start(out=wo_v[1], in_=wbigs[1])
        nc.sync.dma_start(out=xo_v[1], in_=xbigs[1])

    nc.compile()
    return nc


_NC_CACHE = {}


def _get_nc():
    if "nc" not in _NC_CACHE:
        _NC_CACHE["nc"] = _build()
    return _NC_CACHE["nc"]


def _make_in_maps(inputs):
    obs = np.ascontiguousarray(np.asarray(inputs["obs"], np.float32))
    pol = np.ascontiguousarray(np.asarray(inputs["policies"], np.float32))
    act = np.ascontiguousarray(np.asarray(inputs["actions"], np.float32))
    W1 = np.asarray(inputs["W1"], np.float32)
    b1 = np.asarray(inputs["b1"], np.float32)
    W2 = np.asarray(inputs["W2"], np.float32)
    b2 = np.asarray(inputs["b2"], np.float32)
    Wfc = np.asarray(inputs["Wfc"], np.float32)
    Wattn = np.asarray(inputs["Wattn"], np.float32)
    Wv = np.asarray(inputs["Wv"], np.float32)
    bv = np.asarray(inputs["bv"], np.float32)

    wa = (Wfc @ (Wattn[:DZ] + Wattn[DZ:]))[:, 0]     # [64]
    wvy = Wv[DP:, 0]                                  # [8]

    wv64 = Wv[:DP, 0]
    cst = np.zeros((128, CW), np.float32)
    cst[:, 0:8] = wvy[None, :]
    cst[:, 8:72] = W1 / 16.0
    cst[0:64, 72] = W2 @ wa                  # Wq col 0
    cst[0:64, 73] = W2 @ wv64                # Wq col 1
    cst[0:64, 138] = b1
    cst[0, 140] = float(b2 @ wa)             # biasq
    cst[1, 140] = float(b2 @ wv64 + bv[0])
    cst[0:2, 142:144] = np.eye(2, dtype=np.float32)
    cst[:, 144:272] = np.eye(128, dtype=np.float32)

    in_maps = []
    for c in range(NCORES):
        in_maps.append({
            "obs": obs[c * RC:(c + 1) * RC],
            "pol": pol[c * RC:(c + 1) * RC],
            "act": act[c * RC:(c + 1) * RC],
            "cst": cst,
        })
    return in_maps


# Test-harness knobs (the grader just calls kernel() with defaults).
TRACE = False
TRACE_KWARGS = {}
LAST_RESULT = None


def kernel(**inputs):
    global LAST_RESULT
    nc = _get_nc()
    in_maps = _make_in_maps(inputs)
    res = run_bass_kernel_spmd(nc, in_maps, core_ids=list(range(NCORES)),
                               trace=TRACE, **TRACE_KWARGS)
    LAST_RESULT = res
    x = np.concatenate([r["xo"] for r in res.results], axis=0).reshape(B * N, N, 1)
    w = np.concatenate([r["wo"] for r in res.results], axis=0).reshape(B * N, N, 1)
    return x, w
